# revision 17
# baseline (speedup 1.0000x reference)
"""ConvGRU 3-node chain (gnn_message_passing) on 8 TRN2 NeuronCores.

Strategy: pure data parallelism — 1 batch item per core, weights replicated,
no collectives. Per-core kernel: channels-on-partitions, zero-padded 66x66
spatial layout in the SBUF free dimension; every 3x3 conv = 9 shifted matmuls
accumulating in PSUM over row-aligned interior chunks (8 rows x 64 cols =
512); bf16 matmul inputs, fp32 PSUM accumulation; bias + sigmoid/tanh fused
into the PSUM->SBUF drain on the scalar engine; GRU elementwise on vector.

Projection + integrator convs for all three nodes run concurrently as six
64x32 PE sub-tiles (tile_position packing); each gates conv uses the full
128x128 array; the cand convs of nodes 0/1 run as a concurrent 128x64 pair.
"""
import numpy as np

B, T, CIN, H, W = 8, 8, 3, 64, 64
PROJ, CDIM, HID, NUM_NODE = 32, 32, 64, 3
PROCESS_T = T + NUM_NODE - 1  # 10

PW = W + 2                    # padded width 66
IMG = PW * PW                 # 4356
BASEO = 2                     # image offset in the free dim (guard below)
FREE = 4360                   # free size incl guards at both ends
SWEEP_OFF = BASEO + PW        # row-1 col-0 position (GRU elementwise range)
SWEEP_LEN = H * PW            # 4224
NCH = 8                       # chunks per conv: 8 rows x 64 interior cols
RPC = H // NCH                # rows per chunk: 8
TAPS = [di * PW + dj for di in (-1, 0, 1) for dj in (-1, 0, 1)]

N_CORES = 8
_cache = {}


# ------------------------------------------------------------- host packing
def _bf16(x):
    import ml_dtypes
    return np.asarray(x).astype(ml_dtypes.bfloat16)


def _pack_taps(Wc, rows, row_off=0):
    """OIHW conv weight -> [rows, 9*O] bf16 (lhsT blocks, one per tap)."""
    O, I = Wc.shape[0], Wc.shape[1]
    out = np.zeros((rows, 9 * O), np.float32)
    for k in range(9):
        di, dj = k // 3, k % 3
        out[row_off:row_off + I, k * O:(k + 1) * O] = Wc[:, :, di, dj].T
    return _bf16(out)


def _pack_gates(Wg):
    """In-ch order [bu(32); h(64)] -> partition rows [h(64); bu(32)]."""
    Wr = np.concatenate([Wg[:, CDIM:, :, :], Wg[:, :CDIM, :, :]], axis=1)
    return _pack_taps(Wr, 96)


def _prep_inputs(inputs):
    inp = {k: np.asarray(v, np.float32) for k, v in inputs.items()}
    w = {}
    xp = np.zeros((B, PROCESS_T, CIN, H, W), np.float32)
    xp[:, :T] = inp["x"]
    xb = _bf16(xp)

    w["wp0"] = _pack_taps(inp["Win0"], 64)        # x at rows 0-2 (parts 64-66)
    w["wp1"] = _pack_taps(inp["We10"], 64)        # h0 at parts 0-63
    w["wp2"] = _pack_taps(inp["We21"], 64)        # h1 at parts 0-63
    w["wi0"] = _pack_taps(inp["Wint0"], 64, 0)    # p0 rows 0-31 of [p0;p1]
    w["wi1"] = _pack_taps(inp["Wint1"], 64, 32)   # p1 rows 32-63
    w["wi2"] = _pack_taps(inp["Wint2"], 64, 0)    # p2 rows 0-31 (parts 64-95)
    for n in range(3):
        w[f"wg{n}"] = _pack_gates(inp[f"Wg{n}"])
        w[f"wc{n}"] = _pack_gates(inp[f"Wc{n}"])

    bias = np.zeros((128, 12), np.float32)
    for n in range(3):
        bias[:, n] = inp[f"bg{n}"]                # r at 0-63, z at 64-127
        bias[0:64, 3 + n] = inp[f"bc{n}"]
    bias[0:32, 6] = inp["bin0"]
    bias[32:64, 6] = inp["be10"]
    bias[64:96, 6] = inp["be21"]
    bias[0:32, 7] = inp["bint0"]
    bias[96:128, 8] = inp["bint1"]
    bias[32:64, 9] = inp["bint2"]
    bias[64:128, 10] = inp["bc1"]                 # shifted cand1 drain
    return xb, w, bias


# ------------------------------------------------------------ kernel build
def build(n_repeat=1):
    import concourse.bass as bass
    import concourse.bacc as bacc
    import concourse.mybir as mybir
    from concourse import tile

    f32, bf16 = mybir.dt.float32, mybir.dt.bfloat16
    AF = mybir.ActivationFunctionType
    ALU = mybir.AluOpType

    nc = bacc.Bacc(None, target_bir_lowering=False)

    x_ext = nc.declare_dram_parameter("x", [PROCESS_T, CIN, H, W], bf16,
                                      isOutput=False)
    wshapes = {"wp0": (64, 9 * PROJ), "wp1": (64, 9 * PROJ), "wp2": (64, 9 * PROJ),
               "wi0": (64, 9 * CDIM), "wi1": (64, 9 * CDIM), "wi2": (64, 9 * CDIM)}
    for n in range(3):
        wshapes[f"wg{n}"] = (96, 9 * 2 * HID)
        wshapes[f"wc{n}"] = (96, 9 * HID)
    w_ext = {k: nc.declare_dram_parameter(k, list(s), bf16, isOutput=False)
             for k, s in wshapes.items()}
    bias_ext = nc.declare_dram_parameter("bias", [128, 12], f32, isOutput=False)
    out_ext = nc.declare_dram_parameter("out", [HID, H, W], f32, isOutput=True)

    with tile.TileContext(nc) as tc:
        with (
            tc.tile_pool(name="pers", bufs=1) as pers,
            tc.tile_pool(name="ps", bufs=1, space=bass.MemorySpace.PSUM) as ps,
        ):
            def ptile(nm, shape, dt):
                return pers.tile(shape, dt, name=nm, tag=nm, uniquify=False)

            S = [ptile(f"S{n}", [128, FREE], bf16) for n in range(3)]
            C = [ptile(f"C{n}", [128, FREE], bf16) for n in range(3)]
            Z = [ptile(f"Z{n}", [128, FREE], bf16) for n in range(3)]
            D = [ptile(f"D{n}", [128, FREE], bf16) for n in range(3)]
            P = ptile("P", [128, FREE], bf16)
            X = [ptile(f"X{i}", [128, FREE], bf16) for i in range(2)]
            OUTF = ptile("OUTF", [128, H * W], f32)
            WT = {k: ptile(f"w_{k}", [128, wshapes[k][1]], bf16) for k in wshapes}
            BIAS = ptile("BIAS", [128, 12], f32)

            for k in wshapes:
                r0 = 64 if k in ("wp0", "wi2") else 0
                nc.sync.dma_start(WT[k][r0:r0 + wshapes[k][0], :], w_ext[k][:])
            nc.sync.dma_start(BIAS[:], bias_ext[:])
            for tns in [P] + C + Z + D + X:
                nc.gpsimd.memset(tns[:], 0.0)

            def img3(tns, p0, p1):
                return tns[p0:p1, BASEO:BASEO + IMG].rearrange(
                    "p (r s) -> p r s", r=PW, s=PW)

            def mov(tns, p0, p1, c, d):
                """Moving AP for chunk c, tap shift d: [K, 8 rows, 64 cols]."""
                s = BASEO + (1 + RPC * c) * PW + 1 + d
                return tns[p0:p1, s:s + RPC * PW].rearrange(
                    "p (r s) -> p r s", r=RPC, s=PW)[:, :, 0:W]

            def dst(tns, p0, p1, c):
                """Drain destination: interior rows of chunk c."""
                return img3(tns, p0, p1)[:, 1 + RPC * c:1 + RPC * (c + 1), 1:1 + W]

            def q3(q, p0, p1):
                return q[p0:p1, 0:512].rearrange("p (r s) -> p r s", r=RPC, s=W)

            def qtile(nm, tag):
                return ps.tile([128, 512], f32, name=nm, tag=tag, uniquify=True)

            sw = slice(SWEEP_OFF, SWEEP_OFF + SWEEP_LEN)

            for rep in range(n_repeat):
                for n in range(3):
                    nc.gpsimd.memset(S[n][:], 0.0)

                for t in range(PROCESS_T):
                    act1, act2 = t >= 1, t >= 2
                    Xt = X[t % 2]
                    nc.sync.dma_start(img3(Xt, 64, 64 + CIN)[:, 1:1 + H, 1:1 + W],
                                      x_ext[t])

                    def integ_chunk(c):
                        qi0 = qtile(f"q_i0_{rep}_{t}_{c}", "qi0")
                        qi1 = qtile(f"q_i1_{rep}_{t}_{c}", "qi1")
                        qi2 = qtile(f"q_i2_{rep}_{t}_{c}", "qi2") if act2 else None
                        for k in range(9):
                            d = TAPS[k]
                            nc.tensor.matmul(   # integ0: [p0;p1] rows, out parts 0-31
                                qi0[0:32, 0:512], WT["wi0"][0:64, k * 32:k * 32 + 32],
                                mov(P, 0, 64, c, d),
                                start=(k == 0), stop=(k == 8), tile_position=(0, 0))
                            nc.tensor.matmul(   # integ1: out parts 96-127
                                qi1[96:128, 0:512], WT["wi1"][0:64, k * 32:k * 32 + 32],
                                mov(P, 0, 64, c, d),
                                start=(k == 0), stop=(k == 8), tile_position=(0, 96))
                            if act2:
                                nc.tensor.matmul(   # integ2: p2 parts 64-95, out 32-63
                                    qi2[32:64, 0:512],
                                    WT["wi2"][64:128, k * 32:k * 32 + 32],
                                    mov(P, 64, 128, c, d),
                                    start=(k == 0), stop=(k == 8),
                                    tile_position=(64, 32))
                        # drains on vector engine (partition shift to parts 64-95)
                        nc.vector.tensor_scalar_add(
                            dst(S[0], 64, 96, c), q3(qi0, 0, 32), BIAS[0:32, 7:8])
                        nc.vector.tensor_scalar_add(
                            dst(S[1], 64, 96, c), q3(qi1, 96, 128), BIAS[96:128, 8:9])
                        if act2:
                            nc.vector.tensor_scalar_add(
                                dst(S[2], 64, 96, c), q3(qi2, 32, 64), BIAS[32:64, 9:10])

                    # ---------- phase A+B: proj || integ, mode 64x32
                    # (node-2's proj/integ skipped before it activates)
                    for c in range(NCH):
                        q0 = qtile(f"q_p0_{rep}_{t}_{c}", "qp0")
                        q1 = qtile(f"q_p1_{rep}_{t}_{c}", "qp1")
                        q2 = qtile(f"q_p2_{rep}_{t}_{c}", "qp2") if act2 else None
                        for k in range(9):
                            d = TAPS[k]
                            nc.tensor.matmul(   # proj1: h0 -> p1, out parts 32-63
                                q1[32:64, 0:512], WT["wp1"][0:64, k * 32:k * 32 + 32],
                                mov(S[0], 0, 64, c, d),
                                start=(k == 0), stop=(k == 8), tile_position=(0, 32))
                            if act2:
                                nc.tensor.matmul(   # proj2: h1 -> p2, out parts 64-95
                                    q2[64:96, 0:512],
                                    WT["wp2"][0:64, k * 32:k * 32 + 32],
                                    mov(S[1], 0, 64, c, d),
                                    start=(k == 0), stop=(k == 8),
                                    tile_position=(0, 64))
                            nc.tensor.matmul(   # proj0: x -> p0, out parts 0-31
                                q0[0:32, 0:512], WT["wp0"][64:128, k * 32:k * 32 + 32],
                                mov(Xt, 64, 128, c, d),
                                start=(k == 0), stop=(k == 8), tile_position=(64, 0))
                        nc.scalar.activation(dst(P, 0, 32, c), q3(q0, 0, 32),
                                             AF.Identity, bias=BIAS[0:32, 6:7])
                        nc.scalar.activation(dst(P, 32, 64, c), q3(q1, 32, 64),
                                             AF.Identity, bias=BIAS[32:64, 6:7])
                        if act2:
                            nc.scalar.activation(dst(P, 64, 96, c), q3(q2, 64, 96),
                                                 AF.Identity, bias=BIAS[64:96, 6:7])
                        if c >= 2:
                            integ_chunk(c - 2)
                    integ_chunk(NCH - 2)
                    integ_chunk(NCH - 1)

                    # ---------- gates convs, full array, per node
                    # chunks processed in pairs with per-tap bank alternation:
                    # consecutive matmuls into one PSUM bank stall the PE, so
                    # alternate banks (and reuse each tap's stationary twice)
                    for n in range(3):
                        if (n == 1 and not act1) or (n == 2 and not act2):
                            continue
                        for cp in range(0, NCH, 2):
                            qga = qtile(f"q_g{n}_{rep}_{t}_{cp}", "qg0")
                            qgb = qtile(f"q_g{n}_{rep}_{t}_{cp + 1}", "qg1")
                            for k in range(9):
                                d = TAPS[k]
                                for qg, c in ((qga, cp), (qgb, cp + 1)):
                                    nc.tensor.matmul(
                                        qg[0:128, 0:512],
                                        WT[f"wg{n}"][0:96, k * 128:k * 128 + 128],
                                        mov(S[n], 0, 96, c, d),
                                        start=(k == 0), stop=(k == 8),
                                        tile_position=(0, 0))
                            for qg, c in ((qga, cp), (qgb, cp + 1)):
                                nc.scalar.activation(   # r
                                    dst(C[n], 0, 64, c), q3(qg, 0, 64),
                                    AF.Sigmoid, bias=BIAS[0:64, n:n + 1])
                                nc.scalar.activation(   # z: shift 64-127 -> 0-63
                                    dst(Z[n], 0, 64, c), q3(qg, 64, 128),
                                    AF.Sigmoid, bias=BIAS[64:128, n:n + 1])
                                nc.vector.tensor_tensor(   # rh in place
                                    dst(C[n], 0, 64, c), dst(C[n], 0, 64, c),
                                    dst(S[n], 0, 64, c), ALU.mult)
                        nc.vector.tensor_copy(C[n][64:96, sw], S[n][64:96, sw])

                    def cand_pair(ns):
                        """cand convs for the given nodes, interleaved per tap.
                        One node: chunk-paired bank alternation. Two nodes:
                        concurrent col tiles, banks alternate naturally."""
                        specs = []  # (node, colbase, tag)
                        if len(ns) == 2:
                            specs = [(ns[0], 0, "qp0"), (ns[1], 64, "qp1")]
                            for c in range(NCH):
                                qcs = [qtile(f"q_c{n}_{rep}_{t}_{c}", tg)
                                       for n, _, tg in specs]
                                for k in range(9):
                                    d = TAPS[k]
                                    for (n, cb, _), qc in zip(specs, qcs):
                                        nc.tensor.matmul(
                                            qc[cb:cb + 64, 0:512],
                                            WT[f"wc{n}"][0:96, k * 64:k * 64 + 64],
                                            mov(C[n], 0, 96, c, d),
                                            start=(k == 0), stop=(k == 8),
                                            tile_position=(0, cb))
                                for (n, cb, _), qc in zip(specs, qcs):
                                    bcol = 10 if cb == 64 else 3 + n
                                    nc.scalar.activation(
                                        dst(D[n], 0, 64, c), q3(qc, cb, cb + 64),
                                        AF.Tanh, bias=BIAS[cb:cb + 64, bcol:bcol + 1])
                        else:
                            n = ns[0]
                            for cp in range(0, NCH, 2):
                                qca = qtile(f"q_c{n}_{rep}_{t}_{cp}", "qp0")
                                qcb = qtile(f"q_c{n}_{rep}_{t}_{cp + 1}", "qp1")
                                for k in range(9):
                                    d = TAPS[k]
                                    for qc, c in ((qca, cp), (qcb, cp + 1)):
                                        nc.tensor.matmul(
                                            qc[0:64, 0:512],
                                            WT[f"wc{n}"][0:96, k * 64:k * 64 + 64],
                                            mov(C[n], 0, 96, c, d),
                                            start=(k == 0), stop=(k == 8),
                                            tile_position=(0, 0))
                                for qc, c in ((qca, cp), (qcb, cp + 1)):
                                    nc.scalar.activation(
                                        dst(D[n], 0, 64, c), q3(qc, 0, 64),
                                        AF.Tanh, bias=BIAS[0:64, 3 + n:4 + n])

                    def update(n):
                        nc.vector.tensor_tensor(C[n][0:64, sw], S[n][0:64, sw],
                                                D[n][0:64, sw], ALU.subtract)
                        nc.vector.tensor_tensor(C[n][0:64, sw], Z[n][0:64, sw],
                                                C[n][0:64, sw], ALU.mult)
                        nc.vector.tensor_tensor(S[n][0:64, sw], D[n][0:64, sw],
                                                C[n][0:64, sw], ALU.add)

                    cand_pair([0, 1] if act1 else [0])
                    update(0)
                    if act1:
                        update(1)
                    if act2:
                        cand_pair([2])
                        update(2)

                nc.vector.tensor_copy(
                    OUTF[0:64, :].rearrange("p (r s) -> p r s", r=H, s=W),
                    img3(S[2], 0, 64)[:, 1:1 + H, 1:1 + W])
                nc.sync.dma_start(
                    out_ext[:], OUTF[0:64, :].rearrange("p (r s) -> p r s", r=H, s=W))

    nc.compile()
    return nc


# ----------------------------------------------------------------- entry
def kernel(**inputs) -> np.ndarray:
    from concourse.bass_utils import run_bass_kernel_spmd
    xb, w, bias = _prep_inputs(inputs)
    if "nc" not in _cache:
        _cache["nc"] = build(1)
    nc = _cache["nc"]
    in_maps = []
    for b in range(N_CORES):
        m = {"x": np.ascontiguousarray(xb[b]), "bias": bias}
        m.update(w)
        in_maps.append(m)
    res = run_bass_kernel_spmd(nc, in_maps, core_ids=list(range(N_CORES))).results
    return np.stack([res[b]["out"] for b in range(N_CORES)]).astype(np.float32)


# revision 18
# speedup vs baseline: 1.2560x; 1.2560x over previous
"""ConvGRU 3-node chain (gnn_message_passing) on 8 TRN2 NeuronCores.

Strategy: pure data parallelism — 1 batch item per core, weights replicated,
no collectives. Per-core kernel: channels-on-partitions, zero-padded 66x66
spatial layout in the SBUF free dimension; every 3x3 conv = 9 shifted matmuls
accumulating in PSUM over row-aligned interior chunks (8 rows x 64 cols =
512); bf16 matmul inputs, fp32 PSUM accumulation; bias + sigmoid/tanh fused
into the PSUM->SBUF drain on the scalar engine; GRU elementwise on vector.

Projection + integrator convs for all three nodes run concurrently as six
64x32 PE sub-tiles (tile_position packing); each gates conv uses the full
128x128 array; the cand convs of nodes 0/1 run as a concurrent 128x64 pair.
"""
import numpy as np

B, T, CIN, H, W = 8, 8, 3, 64, 64
PROJ, CDIM, HID, NUM_NODE = 32, 32, 64, 3
PROCESS_T = T + NUM_NODE - 1  # 10

PW = W + 2                    # padded width 66
IMG = PW * PW                 # 4356
BASEO = 2                     # image offset in the free dim (guard below)
FREE = 4360                   # free size incl guards at both ends
SWEEP_OFF = BASEO + PW        # row-1 col-0 position (GRU elementwise range)
SWEEP_LEN = H * PW            # 4224
NCH = 8                       # chunks per conv: 8 rows x 64 interior cols
RPC = H // NCH                # rows per chunk: 8
TAPS = [di * PW + dj for di in (-1, 0, 1) for dj in (-1, 0, 1)]

N_CORES = 8
_cache = {}


# ------------------------------------------------------------- host packing
def _bf16(x):
    import ml_dtypes
    return np.asarray(x).astype(ml_dtypes.bfloat16)


def _pack_taps(Wc, rows, row_off=0):
    """OIHW conv weight -> [rows, 9*O] bf16 (lhsT blocks, one per tap)."""
    O, I = Wc.shape[0], Wc.shape[1]
    out = np.zeros((rows, 9 * O), np.float32)
    for k in range(9):
        di, dj = k // 3, k % 3
        out[row_off:row_off + I, k * O:(k + 1) * O] = Wc[:, :, di, dj].T
    return _bf16(out)


def _pack_gates(Wg):
    """In-ch order [bu(32); h(64)] -> partition rows [h(64); bu(32)]."""
    Wr = np.concatenate([Wg[:, CDIM:, :, :], Wg[:, :CDIM, :, :]], axis=1)
    return _pack_taps(Wr, 96)


def _prep_inputs(inputs):
    inp = {k: np.asarray(v, np.float32) for k, v in inputs.items()}
    w = {}
    xp = np.zeros((B, PROCESS_T, CIN, H, W), np.float32)
    xp[:, :T] = inp["x"]
    xb = _bf16(xp)

    w["wp0"] = _pack_taps(inp["Win0"], 64)        # x at rows 0-2 (parts 64-66)
    w["wp1"] = _pack_taps(inp["We10"], 64)        # h0 at parts 0-63
    w["wp2"] = _pack_taps(inp["We21"], 64)        # h1 at parts 0-63
    w["wi0"] = _pack_taps(inp["Wint0"], 64, 0)    # p0 rows 0-31 of [p0;p1]
    w["wi1"] = _pack_taps(inp["Wint1"], 64, 32)   # p1 rows 32-63
    w["wi2"] = _pack_taps(inp["Wint2"], 64, 0)    # p2 rows 0-31 (parts 64-95)
    for n in range(3):
        w[f"wg{n}"] = _pack_gates(inp[f"Wg{n}"])
        w[f"wc{n}"] = _pack_gates(inp[f"Wc{n}"])

    bias = np.zeros((128, 12), np.float32)
    for n in range(3):
        bias[:, n] = inp[f"bg{n}"]                # r at 0-63, z at 64-127
        bias[0:64, 3 + n] = inp[f"bc{n}"]
    bias[0:32, 6] = inp["bin0"]
    bias[32:64, 6] = inp["be10"]
    bias[64:96, 6] = inp["be21"]
    bias[0:32, 7] = inp["bint0"]
    bias[96:128, 8] = inp["bint1"]
    bias[32:64, 9] = inp["bint2"]
    bias[64:128, 10] = inp["bc1"]                 # shifted cand1 drain
    return xb, w, bias


# ------------------------------------------------------------ kernel build
def build(n_repeat=1):
    import concourse.bass as bass
    import concourse.bacc as bacc
    import concourse.mybir as mybir
    from concourse import tile

    f32, bf16 = mybir.dt.float32, mybir.dt.bfloat16
    AF = mybir.ActivationFunctionType
    ALU = mybir.AluOpType

    nc = bacc.Bacc(None, target_bir_lowering=False)

    x_ext = nc.declare_dram_parameter("x", [PROCESS_T, CIN, H, W], bf16,
                                      isOutput=False)
    wshapes = {"wp0": (64, 9 * PROJ), "wp1": (64, 9 * PROJ), "wp2": (64, 9 * PROJ),
               "wi0": (64, 9 * CDIM), "wi1": (64, 9 * CDIM), "wi2": (64, 9 * CDIM)}
    for n in range(3):
        wshapes[f"wg{n}"] = (96, 9 * 2 * HID)
        wshapes[f"wc{n}"] = (96, 9 * HID)
    w_ext = {k: nc.declare_dram_parameter(k, list(s), bf16, isOutput=False)
             for k, s in wshapes.items()}
    bias_ext = nc.declare_dram_parameter("bias", [128, 12], f32, isOutput=False)
    out_ext = nc.declare_dram_parameter("out", [HID, H, W], f32, isOutput=True)

    with tile.TileContext(nc) as tc:
        with (
            tc.tile_pool(name="pers", bufs=1) as pers,
            tc.tile_pool(name="ps", bufs=1, space=bass.MemorySpace.PSUM) as ps,
        ):
            def ptile(nm, shape, dt):
                return pers.tile(shape, dt, name=nm, tag=nm, uniquify=False)

            S = [ptile(f"S{n}", [128, FREE], bf16) for n in range(3)]
            C = [ptile(f"C{n}", [128, FREE], bf16) for n in range(3)]
            Z = [ptile(f"Z{n}", [128, FREE], bf16) for n in range(3)]
            D = [ptile(f"D{n}", [128, FREE], bf16) for n in range(3)]
            P = ptile("P", [128, FREE], bf16)
            X = [ptile(f"X{i}", [128, FREE], bf16) for i in range(2)]
            OUTF = ptile("OUTF", [128, H * W], f32)
            WT = {k: ptile(f"w_{k}", [128, wshapes[k][1]], bf16) for k in wshapes}
            BIAS = ptile("BIAS", [128, 12], f32)

            for k in wshapes:
                r0 = 64 if k in ("wp0", "wi2") else 0
                nc.sync.dma_start(WT[k][r0:r0 + wshapes[k][0], :], w_ext[k][:])
            nc.sync.dma_start(BIAS[:], bias_ext[:])
            for tns in [P] + C + Z + D + X:
                nc.gpsimd.memset(tns[:], 0.0)

            def img3(tns, p0, p1):
                return tns[p0:p1, BASEO:BASEO + IMG].rearrange(
                    "p (r s) -> p r s", r=PW, s=PW)

            def mov(tns, p0, p1, c, d):
                """Moving AP for chunk c, tap shift d: [K, 8 rows, 64 cols]."""
                s = BASEO + (1 + RPC * c) * PW + 1 + d
                return tns[p0:p1, s:s + RPC * PW].rearrange(
                    "p (r s) -> p r s", r=RPC, s=PW)[:, :, 0:W]

            def dst(tns, p0, p1, c):
                """Drain destination: interior rows of chunk c."""
                return img3(tns, p0, p1)[:, 1 + RPC * c:1 + RPC * (c + 1), 1:1 + W]

            def q3(q, p0, p1):
                return q[p0:p1, 0:512].rearrange("p (r s) -> p r s", r=RPC, s=W)

            def qtile(nm, tag):
                return ps.tile([128, 512], f32, name=nm, tag=tag, uniquify=True)

            sw = slice(SWEEP_OFF, SWEEP_OFF + SWEEP_LEN)

            for rep in range(n_repeat):
                for n in range(3):
                    nc.gpsimd.memset(S[n][:], 0.0)

                for t in range(PROCESS_T):
                    act1, act2 = t >= 1, t >= 2
                    Xt = X[t % 2]
                    # x[t] was prefetched during t-1; prefetch x[t+1] now so
                    # the DMA hides behind this timestep's compute
                    if t == 0:
                        nc.sync.dma_start(
                            img3(X[0], 64, 64 + CIN)[:, 1:1 + H, 1:1 + W], x_ext[0])
                    if t + 1 < PROCESS_T:
                        nc.sync.dma_start(
                            img3(X[(t + 1) % 2], 64, 64 + CIN)[:, 1:1 + H, 1:1 + W],
                            x_ext[t + 1])

                    def integ_chunk(c):
                        qi0 = qtile(f"q_i0_{rep}_{t}_{c}", "qi0")
                        qi1 = qtile(f"q_i1_{rep}_{t}_{c}", "qi1")
                        qi2 = qtile(f"q_i2_{rep}_{t}_{c}", "qi2") if act2 else None
                        for k in range(9):
                            d = TAPS[k]
                            nc.tensor.matmul(   # integ0: [p0;p1] rows, out parts 0-31
                                qi0[0:32, 0:512], WT["wi0"][0:64, k * 32:k * 32 + 32],
                                mov(P, 0, 64, c, d),
                                start=(k == 0), stop=(k == 8), tile_position=(0, 0))
                            nc.tensor.matmul(   # integ1: out parts 96-127
                                qi1[96:128, 0:512], WT["wi1"][0:64, k * 32:k * 32 + 32],
                                mov(P, 0, 64, c, d),
                                start=(k == 0), stop=(k == 8), tile_position=(0, 96))
                            if act2:
                                nc.tensor.matmul(   # integ2: p2 parts 64-95, out 32-63
                                    qi2[32:64, 0:512],
                                    WT["wi2"][64:128, k * 32:k * 32 + 32],
                                    mov(P, 64, 128, c, d),
                                    start=(k == 0), stop=(k == 8),
                                    tile_position=(64, 32))
                        # drains on vector engine (partition shift to parts 64-95)
                        nc.vector.tensor_scalar_add(
                            dst(S[0], 64, 96, c), q3(qi0, 0, 32), BIAS[0:32, 7:8])
                        nc.vector.tensor_scalar_add(
                            dst(S[1], 64, 96, c), q3(qi1, 96, 128), BIAS[96:128, 8:9])
                        if act2:
                            nc.vector.tensor_scalar_add(
                                dst(S[2], 64, 96, c), q3(qi2, 32, 64), BIAS[32:64, 9:10])

                    # ---------- phase A+B: proj || integ, mode 64x32
                    # (node-2's proj/integ skipped before it activates)
                    for c in range(NCH):
                        q0 = qtile(f"q_p0_{rep}_{t}_{c}", "qp0")
                        q1 = qtile(f"q_p1_{rep}_{t}_{c}", "qp1")
                        q2 = qtile(f"q_p2_{rep}_{t}_{c}", "qp2") if act2 else None
                        for k in range(9):
                            d = TAPS[k]
                            nc.tensor.matmul(   # proj1: h0 -> p1, out parts 32-63
                                q1[32:64, 0:512], WT["wp1"][0:64, k * 32:k * 32 + 32],
                                mov(S[0], 0, 64, c, d),
                                start=(k == 0), stop=(k == 8), tile_position=(0, 32))
                            if act2:
                                nc.tensor.matmul(   # proj2: h1 -> p2, out parts 64-95
                                    q2[64:96, 0:512],
                                    WT["wp2"][0:64, k * 32:k * 32 + 32],
                                    mov(S[1], 0, 64, c, d),
                                    start=(k == 0), stop=(k == 8),
                                    tile_position=(0, 64))
                            nc.tensor.matmul(   # proj0: x -> p0, out parts 0-31
                                q0[0:32, 0:512], WT["wp0"][64:128, k * 32:k * 32 + 32],
                                mov(Xt, 64, 128, c, d),
                                start=(k == 0), stop=(k == 8), tile_position=(64, 0))
                        nc.scalar.activation(dst(P, 0, 32, c), q3(q0, 0, 32),
                                             AF.Identity, bias=BIAS[0:32, 6:7])
                        nc.scalar.activation(dst(P, 32, 64, c), q3(q1, 32, 64),
                                             AF.Identity, bias=BIAS[32:64, 6:7])
                        if act2:
                            nc.scalar.activation(dst(P, 64, 96, c), q3(q2, 64, 96),
                                                 AF.Identity, bias=BIAS[64:96, 6:7])
                        if c >= 2:
                            integ_chunk(c - 2)
                    integ_chunk(NCH - 2)
                    integ_chunk(NCH - 1)

                    # ---------- gates convs, full array, per node
                    # chunks processed in pairs with per-tap bank alternation:
                    # consecutive matmuls into one PSUM bank stall the PE, so
                    # alternate banks (and reuse each tap's stationary twice)
                    for n in range(3):
                        if (n == 1 and not act1) or (n == 2 and not act2):
                            continue
                        for cp in range(0, NCH, 2):
                            qga = qtile(f"q_g{n}_{rep}_{t}_{cp}", "qg0")
                            qgb = qtile(f"q_g{n}_{rep}_{t}_{cp + 1}", "qg1")
                            for k in range(9):
                                d = TAPS[k]
                                for qg, c in ((qga, cp), (qgb, cp + 1)):
                                    nc.tensor.matmul(
                                        qg[0:128, 0:512],
                                        WT[f"wg{n}"][0:96, k * 128:k * 128 + 128],
                                        mov(S[n], 0, 96, c, d),
                                        start=(k == 0), stop=(k == 8),
                                        tile_position=(0, 0))
                            for qg, c in ((qga, cp), (qgb, cp + 1)):
                                nc.scalar.activation(   # r
                                    dst(C[n], 0, 64, c), q3(qg, 0, 64),
                                    AF.Sigmoid, bias=BIAS[0:64, n:n + 1])
                                nc.scalar.activation(   # z: shift 64-127 -> 0-63
                                    dst(Z[n], 0, 64, c), q3(qg, 64, 128),
                                    AF.Sigmoid, bias=BIAS[64:128, n:n + 1])
                                nc.vector.tensor_tensor(   # rh in place
                                    dst(C[n], 0, 64, c), dst(C[n], 0, 64, c),
                                    dst(S[n], 0, 64, c), ALU.mult)
                        nc.vector.tensor_copy(C[n][64:96, sw], S[n][64:96, sw])

                    def cand_pair(ns):
                        """cand convs for the given nodes, interleaved per tap.
                        One node: chunk-paired bank alternation. Two nodes:
                        concurrent col tiles, banks alternate naturally."""
                        specs = []  # (node, colbase, tag)
                        if len(ns) == 2:
                            specs = [(ns[0], 0, "qp0"), (ns[1], 64, "qp1")]
                            for c in range(NCH):
                                qcs = [qtile(f"q_c{n}_{rep}_{t}_{c}", tg)
                                       for n, _, tg in specs]
                                for k in range(9):
                                    d = TAPS[k]
                                    for (n, cb, _), qc in zip(specs, qcs):
                                        nc.tensor.matmul(
                                            qc[cb:cb + 64, 0:512],
                                            WT[f"wc{n}"][0:96, k * 64:k * 64 + 64],
                                            mov(C[n], 0, 96, c, d),
                                            start=(k == 0), stop=(k == 8),
                                            tile_position=(0, cb))
                                for (n, cb, _), qc in zip(specs, qcs):
                                    bcol = 10 if cb == 64 else 3 + n
                                    nc.scalar.activation(
                                        dst(D[n], 0, 64, c), q3(qc, cb, cb + 64),
                                        AF.Tanh, bias=BIAS[cb:cb + 64, bcol:bcol + 1])
                        else:
                            n = ns[0]
                            for cp in range(0, NCH, 2):
                                qca = qtile(f"q_c{n}_{rep}_{t}_{cp}", "qp0")
                                qcb = qtile(f"q_c{n}_{rep}_{t}_{cp + 1}", "qp1")
                                for k in range(9):
                                    d = TAPS[k]
                                    for qc, c in ((qca, cp), (qcb, cp + 1)):
                                        nc.tensor.matmul(
                                            qc[0:64, 0:512],
                                            WT[f"wc{n}"][0:96, k * 64:k * 64 + 64],
                                            mov(C[n], 0, 96, c, d),
                                            start=(k == 0), stop=(k == 8),
                                            tile_position=(0, 0))
                                for qc, c in ((qca, cp), (qcb, cp + 1)):
                                    nc.scalar.activation(
                                        dst(D[n], 0, 64, c), q3(qc, 0, 64),
                                        AF.Tanh, bias=BIAS[0:64, 3 + n:4 + n])

                    def update(n):
                        nc.vector.tensor_tensor(C[n][0:64, sw], S[n][0:64, sw],
                                                D[n][0:64, sw], ALU.subtract)
                        nc.vector.tensor_tensor(C[n][0:64, sw], Z[n][0:64, sw],
                                                C[n][0:64, sw], ALU.mult)
                        nc.vector.tensor_tensor(S[n][0:64, sw], D[n][0:64, sw],
                                                C[n][0:64, sw], ALU.add)

                    cand_pair([0, 1] if act1 else [0])
                    update(0)
                    if act1:
                        update(1)
                    if act2:
                        cand_pair([2])
                        update(2)

                nc.vector.tensor_copy(
                    OUTF[0:64, :].rearrange("p (r s) -> p r s", r=H, s=W),
                    img3(S[2], 0, 64)[:, 1:1 + H, 1:1 + W])
                nc.sync.dma_start(
                    out_ext[:], OUTF[0:64, :].rearrange("p (r s) -> p r s", r=H, s=W))

    nc.compile()
    return nc


# ----------------------------------------------------------------- entry
def kernel(**inputs) -> np.ndarray:
    from concourse.bass_utils import run_bass_kernel_spmd
    xb, w, bias = _prep_inputs(inputs)
    if "nc" not in _cache:
        _cache["nc"] = build(1)
    nc = _cache["nc"]
    in_maps = []
    for b in range(N_CORES):
        m = {"x": np.ascontiguousarray(xb[b]), "bias": bias}
        m.update(w)
        in_maps.append(m)
    res = run_bass_kernel_spmd(nc, in_maps, core_ids=list(range(N_CORES))).results
    return np.stack([res[b]["out"] for b in range(N_CORES)]).astype(np.float32)
